# revision 44
# baseline (speedup 1.0000x reference)
"""Trainium2 Bass kernel for nn_MultiHeadAttention_7516192768586.

Full MHA: QKV projection -> masked softmax attention -> merge heads ->
residual add -> LayerNorm.  B=2, T=2048, D=1024, 16 heads (depth 64).
Sharding (8 cores): 2 batches x 4 head-groups; each core computes its 4
heads' attention + residual/LN for its 256 output columns; LN statistics
combine via ONE 16KB AllReduce per 4-core batch-group.

Measured 187.9us (cool device) to ~207-211us (after hours of continuous
bench runs -- GPIO thermal throttle drifts; the program is identical).
Baseline was 202.6-219.9us under the same protocol.

Structure (vs the original 202us two-AR baseline):
  * ONE AllReduce for all LN stats, triggered right after the last head's
    stats; all 8 ctx transposes are emitted BEFORE it (Tile serializes DMA
    transposes behind any earlier-emitted collective's completion)
  * warmup collective at the very top: the first collective of a NEFF
    costs ~50-60us here and blocks every later DMA transpose until it
    completes (~75-90us local); chunk-0 stats therefore ride heads whose
    exps all route to the scalar engine (act_mod=1) so a stalled vector
    FIFO cannot starve the PE
  * per-head LN stats ([128,64] STT + accum_out); the tail head's sum-x^2
    half runs on the scalar engine (ACT Square + accum_out) to halve the
    stats latency before the AllReduce trigger
  * rsqrt = Quake seed + ONE Newton step (rel err ~1.7e-3, inside budget)
  * with gamma==1/beta==0 the normalize is (x-mu)*rstd per row: alternated
    2-scalar-op vector tensor_scalar / scalar ACT; general path retained
  * output DMA in 4 streamed chunks; TileContext teardown drops the second
    all-engine barrier (sems are already cleared; saves ~3-5us)
  * ones-only 9th V-proj k-tile replaced by a host den-mask DMA

Steady state (trace): attention heads ~9.3us each, PE ~93% busy within a
head (MM start deltas 250-370ns at N=512); front ~23us is runtime preamble
+ aggregate-DMA-bound; AR 11-17.5us (inter-core launch skew); post-AR tail
~12-15us.

The fp8 DoubleRow mystery (biggest remaining lever, ~20us):
  * PROVEN CORRECT on this HW in isolation AND under kernel-like
    conditions (probe: 4-pair accumulated DR MMs, [128,2,272]-stride lhsT,
    rhs pair planes written on-device by scalar-ACT fp8 exp + vector bf16
    Schraudolph->fp8 copy, both 512-offset rhs slices, all 4 head offsets
    at stride 66 -- matches numpy to the exp-approx tolerance, zero NaN)
  * yet the FULL kernel NaNs scattered whole rows with every variant
    tried: uint8 exp, safe bf16->fp8 exp, all-scalar exp, byte-strided den
    DMA, 2-byte-aligned den DMA, contiguous den DMA + strided vector
    copies.  Eliminated suspects: DR matmul semantics, fp8 ACT exp range,
    den write path, input packing.  Remaining: something in the real-score
    path (pad keys? KTd/QTd-swapped odd tiles?) or a Tile dependency race
    only present in the full pipeline.  Next: run the DR kernel single-core
    in bass_interp.CoreSim with real inputs and diff VH/epairs/cps tensors.

Other measured dead ends (do not redo):
  * DVE float->uint8 convert WRAPS negative values (no saturation)
  * fp8 q/k inputs: rel err 1.04e-2 -> 2.2e-2 (over the 2e-2 gate)
  * input DMAs split across sync+scalar HWDGE queues: -40us (shared BW)
  * partition-major packed input layouts: neutral (front is preamble and
    aggregate-BW bound, not line-size bound)
  * GpSimd/Pool cannot run tensor_scalar/STT on TRN2 (ISA check)
  * batched 4D-gather whole-chunk stats + all-act_mod=2: slower
"""

import sys

if "/opt/trn_rl_repo" not in sys.path:
    sys.path.insert(0, "/opt/trn_rl_repo")

import contextlib

import ml_dtypes
import numpy as np

import bass_rust as _br
import concourse.bass as bass
import concourse.tile as tile
from concourse import mybir
from concourse.bass_utils import run_bass_kernel_spmd
from concourse.vector_clock import ScopedClock

F32 = mybir.dt.float32
BF16 = mybir.dt.bfloat16
FP8 = mybir.dt.float8e4
I16 = mybir.dt.int16
BF = ml_dtypes.bfloat16
E4 = ml_dtypes.float8_e4m3fn

NUM_HEADS = 16
LN_EPS = 1e-5
B, T, D = 2, 2048, 1024
DEPTH = D // NUM_HEADS  # 64
HPC = 4  # heads per core
DD = HPC * DEPTH  # 256 projected channels per core
NKV = D // 128  # 8 v-contraction k-tiles (denominator "ones" row replaced
# by memsets of VH's den columns -- the 9th, ones-only k-tile MM is skipped)
TT = T // 128  # 16 t-tiles
AluOp = mybir.AluOpType
Act = mybir.ActivationFunctionType

# Schraudolph exp on bf16 bit patterns: e^(x/8) ~= bf16_bits(int16(x*ES + EB))
ES = 0.125 * 128.0 * float(np.log2(np.e))
EB = 127.0 * 128.0 - 7.0
# fp8-e4m3 variant with a -ESH shift (softmax is shift-invariant; the shift
# keeps e^(x/8-ESH) within fp8 range): bits = uint8(x*ES8 + EB8); the
# float->uint8 convert saturates at 0, flushing e^(very negative) to +0.0
ESH = 2.0
ES8 = 0.125 * 8.0 * float(np.log2(np.e))
EB8 = 8.0 * (7.0 - ESH * float(np.log2(np.e))) - 0.4375
U8 = mybir.dt.uint8


class _TC(tile.TileContext):
    """TileContext whose tail drain splits its sem waits across 1-wait NOPs
    (this walrus build rejects >1 sync wait on one instruction)."""

    def _drain_and_barrier(self, tick_clock, wait_clock):
        nc = self.nc
        drain_inst = nc.sync.drain()
        wait_clock.add_sem_waits(
            drain_inst.ins, ScopedClock({None: tick_clock.global_clock})
        )
        si = drain_inst.ins.sync_info
        waits = list(si.on_wait) if si is not None and si.on_wait else []
        if len(waits) > 1:
            si.on_wait = waits[:1]
            for i in range(1, len(waits)):
                extra = nc.sync.nop()
                extra.ins.sync_info = _br.SyncInfo(
                    on_wait=waits[i : i + 1], on_update=[]
                )
        nc.all_engine_barrier()
        popped = nc._tile_sem_poison_stack.pop()
        assert popped is self._sem_poison
        assert self.sems is not None
        nc.clear_and_free_semaphores(list(self.sems.allocated().values()))


def _split_multi_waits(nc):
    """Move extra sem waits (>1 per instruction) onto same-engine NOPs
    inserted immediately before the instruction."""
    f = nc.m.functions[0]
    cur_bb = nc.cur_bb
    for block in f.blocks:
        insts = list(block.instructions)
        if not any(
            i.sync_info is not None
            and i.sync_info.on_wait
            and len(i.sync_info.on_wait) > 1
            for i in insts
        ):
            continue
        new_list = []
        for inst in insts:
            si = inst.sync_info
            if si is not None and si.on_wait and len(si.on_wait) > 1:
                waits = list(si.on_wait)
                si.on_wait = waits[:1]
                for w in waits[1:]:
                    eng = nc.engines[inst.engine]
                    nop = eng.nop()
                    tail_bb = cur_bb.bb if hasattr(cur_bb, "bb") else cur_bb
                    tl = list(tail_bb.instructions)
                    assert tl and tl[-1].name == nop.ins.name
                    tail_bb.instructions = tl[:-1]
                    nop.ins.sync_info = _br.SyncInfo(on_wait=[w], on_update=[])
                    new_list.append(nop.ins)
            new_list.append(inst)
        block.instructions = new_list


def _build(SP, trivial_gb):
    """Build the per-core Bass program. SP = padded compacted key count.
    trivial_gb: gamma==1 and beta==0, so the final affine is per-row only."""
    NS = SP // 128  # s-tiles
    NKCH = (SP + 511) // 512  # 512-wide chunks of SP for the K projection

    nc = bass.Bass("TRN2", target_bir_lowering=False, debug=False, num_devices=8)

    p = lambda name, shape, dt: nc.declare_dram_parameter(name, shape, dt, isOutput=False)
    qT = p("qT", [D, T], BF16)
    kT = p("kT", [D, SP], BF16)
    vTs = p("vTs", [NS * D, 128], FP8)  # s-tile-major blocks of v^T
    wqT = p("wqT", [D, DD], BF16)
    wkT = p("wkT", [D, DD], BF16)
    wvT = p("wvT", [D, HPC * (DEPTH + 1)], BF16)
    bq = p("bq", [128, 2], F32)
    bk = p("bk", [128, 2], F32)
    qres = p("qres", [T, DD], BF16)
    gam = p("gamma", [1, DD], BF16)
    bet = p("beta", [1, DD], BF16)
    out = nc.declare_dram_parameter("out", [T, DD], BF16, isOutput=True)

    with _TC(nc) as tc, contextlib.ExitStack() as ctx:
        singles = ctx.enter_context(tc.tile_pool(name="singles", bufs=1))
        persist = ctx.enter_context(tc.tile_pool(name="persist", bufs=1))
        work = ctx.enter_context(tc.tile_pool(name="work", bufs=4))
        dram = ctx.enter_context(tc.tile_pool(name="dram", bufs=1, space="DRAM"))

        # ---- warm the collective path FIRST: the first collective of a NEFF
        # execution pays a ~50-60us setup delay, and Tile serializes every
        # later-emitted DMA transpose behind the preceding collective's
        # completion.  Triggering it at t~0 makes it complete as early as the
        # slowest core's launch allows, so the ctx transposes (needed from
        # ~60us) stall as little as possible.
        warm = singles.tile([128, 1], F32)
        nc.vector.memset(warm[:], 1.0)
        warm_d = dram.tile([128, 1], F32, name="warmd")
        warm_a = dram.tile([128, 1], F32, name="warma")
        nc.sync.dma_start(out=warm_d[:], in_=warm[:])
        nc.gpsimd.collective_compute(
            "AllReduce",
            AluOp.add,
            replica_groups=[[0, 1, 2, 3], [4, 5, 6, 7]],
            ins=[warm_d[:].opt()],
            outs=[warm_a[:].opt()],
        )
        # warm the scalar-engine exp table during the initial DMA wait
        nc.scalar.activation(out=warm[:], in_=warm[:], func=Act.Exp)

        # ---- constants / weights (in consumption order) ----
        wv_sb = singles.tile([128, NKV, HPC * (DEPTH + 1)], BF16)
        nc.sync.dma_start(out=wv_sb[:], in_=wvT[:].rearrange("(kt p) c -> p kt c", p=128))
        bq_sb = singles.tile([128, 2], F32)
        nc.gpsimd.dma_start(out=bq_sb[:], in_=bq[:])
        bk_sb = singles.tile([128, 2], F32)
        nc.gpsimd.dma_start(out=bk_sb[:], in_=bk[:])
        gam_sb = singles.tile([128, DD], BF16)
        g_ap = gam[:]
        nc.gpsimd.dma_start(
            out=gam_sb[:],
            in_=bass.AP(tensor=g_ap.tensor, offset=g_ap.offset, ap=[[0, 128], list(g_ap.ap[-1])]),
        )
        bet_sb = singles.tile([128, DD], BF16)
        b_ap = bet[:]
        nc.gpsimd.dma_start(
            out=bet_sb[:],
            in_=bass.AP(tensor=b_ap.tensor, offset=b_ap.offset, ap=[[0, 128], list(b_ap.ap[-1])]),
        )

        # ---- persistent activations ----
        QT_sb = persist.tile([128, 2, T], BF16)  # [dd-in-tile, ddt, t]
        KT_sb = persist.tile([128, 2, SP], BF16)
        VH_sb = persist.tile([128, NS, HPC * (DEPTH + 1)], BF16)  # [s, st, head*65+c]
        ctxT_sb = persist.tile([128, HPC, T], BF16)  # rows 0..64 valid
        ctxn_sb = persist.tile([128, HPC, TT, 128], BF16)  # transposed ctx
        x_sb = persist.tile([128, TT, DD], BF16)  # residual+ctx, pre-norm
        out_sb = persist.tile([128, TT, DD], BF16)  # normalized output
        mu = singles.tile([128, TT], F32)
        rstd = singles.tile([128, TT], F32)

        # ---- input streaming: all big input DMAs up front on ONE queue in
        # consumption order (vTs, wk, kT, wq, qT, qres).  NB: splitting the
        # inputs across the sync+scalar HWDGE queues was measured WORSE --
        # per-core DMA bandwidth is shared, so a concurrent K/Q stream
        # starves the V operands the first projection is waiting on. ----
        ain = ctx.enter_context(tc.tile_pool(name="ain", bufs=8))
        vin = ctx.enter_context(tc.tile_pool(name="vin", bufs=NS))
        vts = []
        for st in range(NS):
            t_ = vin.tile([128, NKV, 128], FP8, tag="vin", name="vt")
            nc.sync.dma_start(
                out=t_[:],
                in_=vTs[D * st : D * (st + 1), :].rearrange(
                    "(kt p) s -> p kt s", p=128
                ),
            )
            vts.append(t_)
        wk_sb = singles.tile([128, 8, DD], BF16)
        nc.sync.dma_start(out=wk_sb[:], in_=wkT[:].rearrange("(kt p) c -> p kt c", p=128))
        kin = []
        for kt in range(8):
            t_ = ain.tile([128, SP], BF16, tag="kin", name="kin")
            nc.sync.dma_start(out=t_[:], in_=kT[128 * kt : 128 * (kt + 1), :])
            kin.append(t_)
        wq_sb = singles.tile([128, 8, DD], BF16)
        nc.sync.dma_start(out=wq_sb[:], in_=wqT[:].rearrange("(kt p) c -> p kt c", p=128))
        qin = []
        for kt in range(8):
            t_ = ain.tile([128, T], BF16, tag="qin", name="qin")
            nc.sync.dma_start(out=t_[:], in_=qT[128 * kt : 128 * (kt + 1), :])
            qin.append(t_)
        qres_sb = persist.tile([128, TT, DD], BF16)
        nc.sync.dma_start(
            out=qres_sb[:], in_=qres[:].rearrange("(tt p) c -> p tt c", p=128)
        )

        # host-prepared per-key softmax-denominator mask (1.0 for valid keys,
        # 0.0 for compaction padding), DMA'd into VH's den columns after the
        # projection copies (the wv den columns are zero, so PSUM den = 0)
        denm = p("denm", [128, NS * HPC], BF16)
        denm_dst = bass.AP(
            tensor=VH_sb.tensor,
            offset=VH_sb[:, 0, DEPTH].offset,
            ap=[VH_sb.ap[0], [HPC * (DEPTH + 1), NS], [DEPTH + 1, HPC]],
        )

        # ---- V projection (own PSUM scope, closed before attention) ----
        with tc.tile_pool(name="pv", bufs=2, space="PSUM") as pv:
            for st in range(NS):
                ps = pv.tile([128, HPC * (DEPTH + 1)], F32, tag="pv")
                for kt in range(NKV):
                    nc.tensor.matmul(
                        ps[:],
                        vts[st][:, kt, :],
                        wv_sb[:, kt, :],
                        start=(kt == 0),
                        stop=(kt == NKV - 1),
                    )
                if st % 2 == 0:
                    nc.vector.tensor_copy(VH_sb[:, st, :], ps[:])
                else:
                    nc.scalar.copy(VH_sb[:, st, :], ps[:])
        nc.gpsimd.dma_start(out=denm_dst, in_=denm[:])

        # ---- K/Q projections (own PSUM scope) ----
        with tc.tile_pool(name="pqk", bufs=3, space="PSUM") as pqk:

            def k_proj(ddt):
                for sch in range(NKCH):
                    w = min(512, SP - 512 * sch)
                    ps = pqk.tile([128, 512], F32, tag="pqk", name="kps")
                    for kt in range(8):
                        nc.tensor.matmul(
                            ps[:, :w],
                            wk_sb[:, kt, 128 * ddt : 128 * (ddt + 1)],
                            kin[kt][:, 512 * sch : 512 * sch + w],
                            start=(kt == 0),
                            stop=(kt == 7),
                        )
                    dst = KT_sb[:, ddt, 512 * sch : 512 * sch + w]
                    if sch % 2 == 0:
                        nc.vector.tensor_scalar(
                            out=dst, in0=ps[:, :w],
                            scalar1=bk_sb[:, ddt : ddt + 1], scalar2=None,
                            op0=AluOp.add,
                        )
                    else:
                        nc.scalar.activation(
                            out=dst, in_=ps[:, :w], func=Act.Identity,
                            bias=bk_sb[:, ddt : ddt + 1], scale=1.0,
                        )

            def q_proj(ddt):
                for tch in range(4):
                    ps = pqk.tile([128, 512], F32, tag="pqk", name="qps")
                    for kt in range(8):
                        nc.tensor.matmul(
                            ps[:],
                            wq_sb[:, kt, 128 * ddt : 128 * (ddt + 1)],
                            qin[kt][:, 512 * tch : 512 * (tch + 1)],
                            start=(kt == 0),
                            stop=(kt == 7),
                        )
                    dst = QT_sb[:, ddt, 512 * tch : 512 * (tch + 1)]
                    if tch % 2 == 0:
                        nc.scalar.activation(
                            out=dst, in_=ps[:], func=Act.Identity,
                            bias=bq_sb[:, ddt : ddt + 1], scale=1.0,
                        )
                    else:
                        nc.vector.tensor_scalar(
                            out=dst, in0=ps[:],
                            scalar1=bq_sb[:, ddt : ddt + 1], scalar2=None,
                            op0=AluOp.add,
                        )

            k_proj(0)
            q_proj(0)
            k_proj(1)
            q_proj(1)

        # partition-swapped copies of KT/QT: row-group 0-63 holds the odd
        # head's depth rows and 64-127 the even head's, so consecutive
        # s-tiles' K=64 score matmuls can run concurrently in disjoint
        # PE row groups (SBUF->SBUF DMA shifts partitions; engines can't)
        KTd_sb = persist.tile([128, 2, SP], BF16)
        QTd_sb = persist.tile([128, 2, T], BF16)
        for ddt in range(2):
            nc.sync.dma_start(
                out=KTd_sb[64:128, ddt, :], in_=KT_sb[0:64, ddt, :])
            nc.sync.dma_start(
                out=KTd_sb[0:64, ddt, :], in_=KT_sb[64:128, ddt, :])
            nc.sync.dma_start(
                out=QTd_sb[64:128, ddt, :], in_=QT_sb[0:64, ddt, :])
            nc.sync.dma_start(
                out=QTd_sb[0:64, ddt, :], in_=QT_sb[64:128, ddt, :])

        # ---- attention + pipelined LN ----
        stats_dram = dram.tile([128, 32], F32, name="stats")
        ar_dram = dram.tile([128, 32], F32, name="ar")
        epool = ctx.enter_context(tc.tile_pool(name="epool", bufs=6))
        # PSUM: 3 score tiles (2 banks each, depth-3 pipeline) + 1 ctx (2) = 8
        pscore = ctx.enter_context(tc.tile_pool(name="pscore", bufs=3, space="PSUM"))
        pctx = ctx.enter_context(tc.tile_pool(name="pctx", bufs=1, space="PSUM"))

        # exp engine schedule: every act_mod-th tile on the scalar engine
        # (exact), the rest on the vector engine (bit-trick)
        ecnt = [0]

        def emit_exp(e_ap, s_ap, act_mod):
            i = ecnt[0]
            ecnt[0] += 1
            if i % act_mod == act_mod - 1:
                nc.scalar.activation(out=e_ap, in_=s_ap, func=Act.Exp, scale=0.125)
            else:
                nc.vector.tensor_scalar(
                    out=e_ap.bitcast(I16), in0=s_ap,
                    scalar1=ES, scalar2=EB,
                    op0=AluOp.mult, op1=AluOp.add,
                )

        def attn_head(hd, tc_, copy_eng, act_mod=3, fillers=()):
            """Attention for head hd over t-chunk tc_ (1024 queries).

            Software-pipelined: scores(st) and exp(st) are emitted before
            ctx(st-1) so the PE streams scores while ACT/DVE exponentiate
            and the 3-deep score-psum pool absorbs exp latency jitter.
            ``fillers``: closures spread evenly across the head's s-tile
            pair iterations (LN stats work threads through the attention
            instruction stream)."""
            t0 = 1024 * tc_
            ddt, h = hd // 2, hd % 2
            r0 = DEPTH * h
            cps = pctx.tile([DEPTH + 1, 1024], F32, tag="cps", name="cps")
            es = [None] * NS
            fillers = list(fillers)
            npairs = (NS + 1) // 2

            def emit_ctx(st):
                for q2 in range(2):
                    nc.tensor.matmul(
                        cps[:, 512 * q2 : 512 * (q2 + 1)],
                        VH_sb[:, st, (DEPTH + 1) * hd : (DEPTH + 1) * (hd + 1)],
                        es[st][:, 512 * q2 : 512 * (q2 + 1)],
                        start=(st == 0),
                        stop=(st == NS - 1),
                    )

            # odd s-tiles use the partition-swapped K/Q copies so their
            # K=64 score matmuls land in the opposite PE row group and run
            # concurrently with the even s-tile's
            rd0 = DEPTH * (1 - h)
            for it, sp in enumerate(range(0, NS, 2)):
                pair = sp + 1 < NS
                s0 = pscore.tile([128, 1024], F32, tag="sc", name="s")
                s1 = pscore.tile([128, 1024], F32, tag="sc", name="s") if pair else None
                for q2 in range(2):
                    nc.tensor.matmul(
                        s0[:, 512 * q2 : 512 * (q2 + 1)],
                        KT_sb[r0 : r0 + DEPTH, ddt, 128 * sp : 128 * (sp + 1)],
                        QT_sb[r0 : r0 + DEPTH, ddt, t0 + 512 * q2 : t0 + 512 * (q2 + 1)],
                        start=True,
                        stop=True,
                    )
                    if pair:
                        nc.tensor.matmul(
                            s1[:, 512 * q2 : 512 * (q2 + 1)],
                            KTd_sb[rd0 : rd0 + DEPTH, ddt, 128 * (sp + 1) : 128 * (sp + 2)],
                            QTd_sb[rd0 : rd0 + DEPTH, ddt, t0 + 512 * q2 : t0 + 512 * (q2 + 1)],
                            start=True,
                            stop=True,
                        )
                e_ = epool.tile([128, 1024], BF16, tag="e")
                emit_exp(e_[:], s0[:], act_mod)
                es[sp] = e_
                if pair:
                    e2_ = epool.tile([128, 1024], BF16, tag="e")
                    emit_exp(e2_[:], s1[:], act_mod)
                    es[sp + 1] = e2_
                if sp > 0:
                    emit_ctx(sp - 2)
                    if sp - 1 < NS:
                        emit_ctx(sp - 1)
                # spread all fillers evenly over the remaining iterations
                take = -(-len(fillers) // (npairs - it)) if fillers else 0
                for _ in range(take):
                    fillers.pop(0)()
            for st_ in range(2 * ((NS - 1) // 2), NS):
                emit_ctx(st_)
            for f in fillers:
                f()

            dst = ctxT_sb[0 : DEPTH + 1, hd, t0 : t0 + 1024]
            if copy_eng == "v":
                nc.vector.tensor_copy(dst, cps[:])
            else:
                nc.scalar.copy(dst, cps[:])
            nc.sync.dma_start_transpose(
                ctxn_sb[:, hd, 8 * tc_ : 8 * (tc_ + 1), :],
                ctxT_sb[:, hd, t0 : t0 + 1024],
            )

        ln = ctx.enter_context(tc.tile_pool(name="ln", bufs=4))

        hs_t = [
            singles.tile([128, HPC, 16], F32, tag=f"hs{i}", name=f"hs{i}")
            for i in range(2)
        ]
        rinv_t = {}

        def phase5_rinv(tc_, h):
            """Reciprocal of head (tc_, h)'s softmax denominators (vector).
            Must be EMITTED before any phase5_head closures for (tc_, h)."""
            rinv_h = work.tile([128, 8], F32, tag=f"rh{tc_}_{h}", name=f"rh{tc_}_{h}")
            rinv_t[tc_, h] = rinv_h
            sums_ap = bass.AP(
                tensor=ctxn_sb.tensor,
                offset=ctxn_sb[:, h, 8 * tc_, DEPTH].offset,
                ap=[ctxn_sb.ap[0], [128, 8], [1, 1]],
            )
            nc.vector.reciprocal(rinv_h[:], sums_ap)

        def phase5_head(tc_, h, js=range(8), with_rinv=True, sxx_eng="v"):
            """Stats closures for head (tc_, h)'s 64 columns (stats are
            additive over heads).  sxx_eng="s" computes the sum-x^2 half on
            the scalar engine (ACT Square + accum_out)."""
            hs1 = hs_t[tc_]
            cls = []
            if with_rinv:
                cls.append(lambda: phase5_rinv(tc_, h))
            for j in js:
                tt = 8 * tc_ + j
                c0 = DEPTH * h

                def sx(tt=tt, j=j, c0=c0, h=h):
                    nc.vector.scalar_tensor_tensor(
                        out=x_sb[:, tt, c0 : c0 + DEPTH],
                        in0=ctxn_sb[:, h, tt, 0:DEPTH],
                        scalar=rinv_t[tc_, h][:, j : j + 1],
                        in1=qres_sb[:, tt, c0 : c0 + DEPTH],
                        op0=AluOp.mult, op1=AluOp.add,
                        accum_out=hs1[:, h, j : j + 1],
                    )

                def sxx(tt=tt, j=j, c0=c0, h=h):
                    xx = ln.tile([128, DEPTH], BF16, tag="hxx")
                    if sxx_eng == "s":
                        nc.scalar.activation(
                            out=xx[:], in_=x_sb[:, tt, c0 : c0 + DEPTH],
                            func=Act.Square,
                            accum_out=hs1[:, h, 8 + j : 9 + j],
                        )
                    else:
                        nc.vector.scalar_tensor_tensor(
                            out=xx[:], in0=x_sb[:, tt, c0 : c0 + DEPTH], scalar=0.0,
                            in1=x_sb[:, tt, c0 : c0 + DEPTH],
                            op0=AluOp.add, op1=AluOp.mult,
                            accum_out=hs1[:, h, 8 + j : 9 + j],
                        )

                cls += [sx, sxx]
            return cls

        stats32 = singles.tile([128, 32], F32, name="stats32")

        def phase5_merge(tc_):
            """stats32[16*tc_:16*tc_+16] = sum over heads of per-head stats
            (cols 0:8 sum-x, 8:16 sum-x^2)."""
            hs1 = hs_t[tc_]
            merged = bass.AP(
                tensor=hs1.tensor,
                offset=hs1[:, 0, 0].offset,
                ap=[hs1.ap[0], [1, 16], [16, HPC]],
            )
            nc.vector.tensor_reduce(
                out=stats32[:, 16 * tc_ : 16 * tc_ + 16], in_=merged,
                axis=mybir.AxisListType.X, op=AluOp.add,
            )

        def phase5_cc():
            # single 16KB AllReduce for ALL LayerNorm stats; every DMA
            # transpose is emitted before this point so none serializes
            # behind it
            nc.sync.dma_start(out=stats_dram[:, 0:32], in_=stats32[:])
            nc.gpsimd.collective_compute(
                "AllReduce",
                AluOp.add,
                replica_groups=[[0, 1, 2, 3], [4, 5, 6, 7]],
                ins=[stats_dram[:, 0:32].opt()],
                outs=[ar_dram[:, 0:32].opt()],
            )

        def phase7_prep():
            """After the AllReduce: mu and rstd for all 16 t-tiles on the
            vector engine (rsqrt = Quake bit-trick seed + 2 Newton steps)."""
            gst = work.tile([128, 32], F32, tag="gst", name="gst")
            nc.sync.dma_start(out=gst[:], in_=ar_dram[:, 0:32])
            # strided views: sums at cols {0:8, 16:24}, sumsq at {8:16, 24:32}
            sums = bass.AP(tensor=gst.tensor, offset=gst[:, 0].offset,
                           ap=[gst.ap[0], [16, 2], [1, 8]])
            sumsq = bass.AP(tensor=gst.tensor, offset=gst[:, 8].offset,
                            ap=[gst.ap[0], [16, 2], [1, 8]])
            nc.vector.tensor_scalar(
                out=mu[:, 0:16], in0=sums, scalar1=1.0 / D, scalar2=None,
                op0=AluOp.mult,
            )
            var = work.tile([128, 16], F32, tag="var", name="var")
            nc.vector.tensor_tensor(out=var[:], in0=mu[:, 0:16], in1=mu[:, 0:16],
                                    op=AluOp.mult)
            nc.vector.scalar_tensor_tensor(
                out=var[:], in0=sumsq, scalar=1.0 / D, in1=var[:],
                op0=AluOp.mult, op1=AluOp.subtract,
            )
            nc.vector.tensor_scalar(
                out=var[:], in0=var[:], scalar1=LN_EPS, scalar2=None, op0=AluOp.add
            )
            y = work.tile([128, 16], F32, tag="y", name="y")
            nc.vector.tensor_scalar(
                out=y[:].bitcast(mybir.dt.int32),
                in0=var[:].bitcast(mybir.dt.int32),
                scalar1=-0.5, scalar2=float(0x5F3759DF),
                op0=AluOp.mult, op1=AluOp.add,
            )
            tn = work.tile([128, 16], F32, tag="tn", name="tn")
            for it in range(1):
                nc.vector.tensor_tensor(out=tn[:], in0=var[:], in1=y[:], op=AluOp.mult)
                nc.vector.tensor_tensor(out=tn[:], in0=tn[:], in1=y[:], op=AluOp.mult)
                nc.vector.tensor_scalar(
                    out=tn[:], in0=tn[:], scalar1=-0.5, scalar2=1.5,
                    op0=AluOp.mult, op1=AluOp.add,
                )
                nc.vector.tensor_tensor(out=rstd[:, 0:16], in0=y[:], in1=tn[:],
                                        op=AluOp.mult)
            if trivial_gb:
                # out = (x - mu) * rstd needs only a per-row scale+bias:
                # one scalar-engine ACT per tile.  negmurstd = -mu * rstd.
                nc.vector.scalar_tensor_tensor(
                    out=negmurstd[:], in0=mu[:, 0:16], scalar=-1.0,
                    in1=rstd[:, 0:16], op0=AluOp.mult, op1=AluOp.mult,
                )

        negmurstd = work.tile([128, 16], F32, tag="nmr", name="nmr")

        def phase7_g(g):
            """Normalize 4 t-tiles.  With gamma==1/beta==0 the whole affine
            collapses to per-row (x - mu) * rstd: one 2-scalar-op vector
            tensor_scalar OR one scalar-engine ACT per tile, alternated so
            both engines split the tail; otherwise two vector STTs."""
            for k in range(4):
                tt = 4 * g + k
                if trivial_gb:
                    if k % 2 == 0:
                        nc.vector.tensor_scalar(
                            out=out_sb[:, tt, :], in0=x_sb[:, tt, :],
                            scalar1=mu[:, tt : tt + 1],
                            scalar2=rstd[:, tt : tt + 1],
                            op0=AluOp.subtract, op1=AluOp.mult,
                        )
                    else:
                        nc.scalar.activation(
                            out=out_sb[:, tt, :], in_=x_sb[:, tt, :],
                            func=Act.Identity,
                            bias=negmurstd[:, tt : tt + 1],
                            scale=rstd[:, tt : tt + 1],
                        )
                    continue
                s1 = ln.tile([128, DD], BF16, tag="s1")
                nc.vector.scalar_tensor_tensor(
                    out=s1[:], in0=x_sb[:, tt, :], scalar=mu[:, tt : tt + 1],
                    in1=gam_sb[:], op0=AluOp.subtract, op1=AluOp.mult,
                )
                nc.vector.scalar_tensor_tensor(
                    out=out_sb[:, tt, :], in0=s1[:], scalar=rstd[:, tt : tt + 1],
                    in1=bet_sb[:], op0=AluOp.mult, op1=AluOp.add,
                )

        def phase7_out(g):
            nc.sync.dma_start(
                out=out[512 * g : 512 * (g + 1), :].rearrange(
                    "(tt p) c -> p tt c", p=128
                ),
                in_=out_sb[:, 4 * g : 4 * (g + 1), :],
            )

        # Schedule: all 8 heads' LN stats thread through the attention
        # stream on the vector engine.  Chunk-0 stats wait on ctx transposes
        # that are serialized behind the warmup collective (~70-90us local);
        # while those emissions can stall the vector FIFO, every exp routes
        # to the scalar engine (act_mod=1).  A single AllReduce of all stats
        # triggers right after the last head's stats; the tail is prep +
        # scalar-engine normalize + streamed output DMAs.
        attn_head(0, 0, "s", act_mod=2)
        attn_head(1, 0, "v", act_mod=2)
        phase5_rinv(0, 0)
        for cl in phase5_head(0, 0, with_rinv=False):
            cl()
        attn_head(2, 0, "s", act_mod=1)
        phase5_rinv(0, 1)
        for cl in phase5_head(0, 1, with_rinv=False):
            cl()
        attn_head(3, 0, "s", act_mod=1)
        phase5_rinv(0, 2)
        for cl in phase5_head(0, 2, with_rinv=False):
            cl()
        attn_head(0, 1, "s", act_mod=1)
        phase5_rinv(0, 3)
        for cl in phase5_head(0, 3, with_rinv=False):
            cl()
        attn_head(1, 1, "s", act_mod=1)
        phase5_rinv(1, 0)
        for cl in phase5_head(1, 0, with_rinv=False):
            cl()
        attn_head(2, 1, "v", act_mod=2, fillers=[lambda: phase5_merge(0)])
        phase5_rinv(1, 1)
        for cl in phase5_head(1, 1, with_rinv=False):
            cl()
        phase5_rinv(1, 2)
        for cl in phase5_head(1, 2, with_rinv=False):
            cl()
        attn_head(3, 1, "v", act_mod=1)
        phase5_rinv(1, 3)
        for cl in phase5_head(1, 3, with_rinv=False, sxx_eng="s"):
            cl()
        phase5_merge(1)
        phase5_cc()
        phase7_prep()
        for g in range(4):
            phase7_g(g)
            phase7_out(g)

    _split_multi_waits(nc)
    return nc


_CACHE = {}
_LAST_IN_MAPS = None


def kernel(q, k, v, mask, causality, edge_fea, wq, bq, wk, bk, wv, bv, gamma, beta):
    # NB: the reference masks attention row (head eta, batch beta) with
    # mask[eta // 8]; with 4 heads per core this is mask[hg // 2].
    q = np.asarray(q, np.float32)
    k = np.asarray(k, np.float32)
    v = np.asarray(v, np.float32)
    mask = np.asarray(mask)
    wq = np.asarray(wq, np.float32)
    bq = np.asarray(bq, np.float32)
    wk = np.asarray(wk, np.float32)
    bk = np.asarray(bk, np.float32)
    wv = np.asarray(wv, np.float32)
    bv = np.asarray(bv, np.float32)
    gamma = np.asarray(gamma, np.float32)
    beta = np.asarray(beta, np.float32)
    assert int(np.asarray(causality)) == 0

    keep = [np.flatnonzero(mask[g] == 0) for g in range(2)]
    slens = [len(kp) for kp in keep]
    SP = max(128, ((max(slens) + 127) // 128) * 128)
    NS = SP // 128

    qT = [np.ascontiguousarray(q[b].T).astype(BF) for b in range(2)]
    kTc, vTc, denc = {}, {}, {}
    for b in range(2):
        for g in range(2):
            kk = np.zeros((D, SP), BF)
            kk[:, : slens[g]] = k[b][keep[g]].T.astype(BF)
            kTc[b, g] = kk
            vv = np.zeros((D, SP), E4)
            vv[:, : slens[g]] = v[b][keep[g]].T.astype(E4)
            # s-tile-major blocks: [NS, D, 128] -> [NS*D, 128]
            vTc[b, g] = np.ascontiguousarray(
                vv.reshape(D, NS, 128).transpose(1, 0, 2).reshape(NS * D, 128)
            )
    for g in range(2):
        dm = np.zeros((128, NS, HPC), BF)
        valid = (np.arange(SP) < slens[g]).reshape(NS, 128).T  # [128, NS]
        dm[:, :, :] = valid[:, :, None].astype(BF)
        denc[g] = np.ascontiguousarray(dm.reshape(128, NS * HPC))

    in_maps = []
    for c in range(8):
        b, hg = c // 4, c % 4
        g = hg // 2
        c0 = hg * DD
        wvp = np.zeros((D, HPC * (DEPTH + 1)), BF)
        for hh in range(HPC):
            wvp[:, hh * (DEPTH + 1) : hh * (DEPTH + 1) + DEPTH] = (
                wv[c0 + hh * DEPTH : c0 + (hh + 1) * DEPTH].T.astype(BF)
            )
        in_maps.append(
            {
                "qT": qT[b],
                "kT": kTc[b, g],
                "vTs": vTc[b, g],
                "wqT": np.ascontiguousarray(wq[c0 : c0 + DD].T).astype(BF),
                "wkT": np.ascontiguousarray(wk[c0 : c0 + DD].T).astype(BF),
                "wvT": wvp,
                "denm": denc[g],
                "bq": np.ascontiguousarray(bq[c0 : c0 + DD].reshape(2, 128).T),
                "bk": np.ascontiguousarray(bk[c0 : c0 + DD].reshape(2, 128).T),
                "qres": (q[b][:, c0 : c0 + DD] + bv[c0 : c0 + DD]).astype(BF),
                "gamma": gamma[c0 : c0 + DD].reshape(1, DD).astype(BF),
                "beta": beta[c0 : c0 + DD].reshape(1, DD).astype(BF),
            }
        )

    global _LAST_IN_MAPS
    _LAST_IN_MAPS = in_maps
    trivial_gb = bool(np.all(gamma == 1.0) and np.all(beta == 0.0))
    key = (SP, trivial_gb)
    if key not in _CACHE:
        _CACHE[key] = _build(SP, trivial_gb)
    nc = _CACHE[key]

    res = run_bass_kernel_spmd(nc, in_maps, list(range(8))).results

    full = np.empty((B, T, D), np.float32)
    for c in range(8):
        b, hg = c // 4, c % 4
        full[b, :, hg * DD : (hg + 1) * DD] = np.asarray(res[c]["out"], np.float32)
    return full



# revision 45
# speedup vs baseline: 1.0555x; 1.0555x over previous
"""Trainium2 Bass kernel for nn_MultiHeadAttention_7516192768586.

Full MHA: QKV projection -> masked softmax attention -> merge heads ->
residual add -> LayerNorm.  B=2, T=2048, D=1024, 16 heads (depth 64).
Sharding (8 cores): 2 batches x 4 head-groups; each core computes its 4
heads' attention + residual/LN for its 256 output columns; LN statistics
combine via ONE 16KB AllReduce per 4-core batch-group.

Measured 187.9us (cool device) to ~207-211us (after hours of continuous
bench runs -- GPIO thermal throttle drifts; the program is identical).
Baseline was 202.6-219.9us under the same protocol.

Structure (vs the original 202us two-AR baseline):
  * ONE AllReduce for all LN stats, triggered right after the last head's
    stats; all 8 ctx transposes are emitted BEFORE it (Tile serializes DMA
    transposes behind any earlier-emitted collective's completion)
  * warmup collective at the very top: the first collective of a NEFF
    costs ~50-60us here and blocks every later DMA transpose until it
    completes (~75-90us local); chunk-0 stats therefore ride heads whose
    exps all route to the scalar engine (act_mod=1) so a stalled vector
    FIFO cannot starve the PE
  * per-head LN stats ([128,64] STT + accum_out); the tail head's sum-x^2
    half runs on the scalar engine (ACT Square + accum_out) to halve the
    stats latency before the AllReduce trigger
  * rsqrt = Quake seed + ONE Newton step (rel err ~1.7e-3, inside budget)
  * with gamma==1/beta==0 the normalize is (x-mu)*rstd per row: alternated
    2-scalar-op vector tensor_scalar / scalar ACT; general path retained
  * output DMA in 4 streamed chunks; TileContext teardown drops the second
    all-engine barrier (sems are already cleared; saves ~3-5us)
  * ones-only 9th V-proj k-tile replaced by a host den-mask DMA

Steady state (trace): attention heads ~9.3us each, PE ~93% busy within a
head (MM start deltas 250-370ns at N=512); front ~23us is runtime preamble
+ aggregate-DMA-bound; AR 11-17.5us (inter-core launch skew); post-AR tail
~12-15us.

The fp8 DoubleRow mystery (biggest remaining lever, ~20us):
  * PROVEN CORRECT on this HW in isolation AND under kernel-like
    conditions (probe: 4-pair accumulated DR MMs, [128,2,272]-stride lhsT,
    rhs pair planes written on-device by scalar-ACT fp8 exp + vector bf16
    Schraudolph->fp8 copy, both 512-offset rhs slices, all 4 head offsets
    at stride 66 -- matches numpy to the exp-approx tolerance, zero NaN)
  * yet the FULL kernel NaNs scattered whole rows with every variant
    tried: uint8 exp, safe bf16->fp8 exp, all-scalar exp, byte-strided den
    DMA, 2-byte-aligned den DMA, contiguous den DMA + strided vector
    copies.  Eliminated suspects: DR matmul semantics, fp8 ACT exp range,
    den write path, input packing.  Remaining: something in the real-score
    path (pad keys? KTd/QTd-swapped odd tiles?) or a Tile dependency race
    only present in the full pipeline.  Next: run the DR kernel single-core
    in bass_interp.CoreSim with real inputs and diff VH/epairs/cps tensors.

Other measured dead ends (do not redo):
  * DVE float->uint8 convert WRAPS negative values (no saturation)
  * fp8 q/k inputs: rel err 1.04e-2 -> 2.2e-2 (over the 2e-2 gate)
  * input DMAs split across sync+scalar HWDGE queues: -40us (shared BW)
  * partition-major packed input layouts: neutral (front is preamble and
    aggregate-BW bound, not line-size bound)
  * GpSimd/Pool cannot run tensor_scalar/STT on TRN2 (ISA check)
  * batched 4D-gather whole-chunk stats + all-act_mod=2: slower
"""

import sys

if "/opt/trn_rl_repo" not in sys.path:
    sys.path.insert(0, "/opt/trn_rl_repo")

import contextlib

import ml_dtypes
import numpy as np

import bass_rust as _br
import concourse.bass as bass
import concourse.tile as tile
from concourse import mybir
from concourse.bass_utils import run_bass_kernel_spmd
from concourse.vector_clock import ScopedClock

F32 = mybir.dt.float32
BF16 = mybir.dt.bfloat16
FP8 = mybir.dt.float8e4
I16 = mybir.dt.int16
BF = ml_dtypes.bfloat16
E4 = ml_dtypes.float8_e4m3fn

NUM_HEADS = 16
LN_EPS = 1e-5
B, T, D = 2, 2048, 1024
DEPTH = D // NUM_HEADS  # 64
HPC = 4  # heads per core
DD = HPC * DEPTH  # 256 projected channels per core
NKV = D // 128  # 8 v-contraction k-tiles (denominator "ones" row replaced
# by memsets of VH's den columns -- the 9th, ones-only k-tile MM is skipped)
TT = T // 128  # 16 t-tiles
AluOp = mybir.AluOpType
Act = mybir.ActivationFunctionType

# Schraudolph exp on bf16 bit patterns: e^(x/8) ~= bf16_bits(int16(x*ES + EB))
ES = 0.125 * 128.0 * float(np.log2(np.e))
EB = 127.0 * 128.0 - 7.0
# fp8-e4m3 variant with a -ESH shift (softmax is shift-invariant; the shift
# keeps e^(x/8-ESH) within fp8 range): bits = uint8(x*ES8 + EB8); the
# float->uint8 convert saturates at 0, flushing e^(very negative) to +0.0
ESH = 2.0
ES8 = 0.125 * 8.0 * float(np.log2(np.e))
EB8 = 8.0 * (7.0 - ESH * float(np.log2(np.e))) - 0.4375
U8 = mybir.dt.uint8


class _TC(tile.TileContext):
    """TileContext whose tail drain splits its sem waits across 1-wait NOPs
    (this walrus build rejects >1 sync wait on one instruction)."""

    def _drain_and_barrier(self, tick_clock, wait_clock):
        nc = self.nc
        drain_inst = nc.sync.drain()
        wait_clock.add_sem_waits(
            drain_inst.ins, ScopedClock({None: tick_clock.global_clock})
        )
        si = drain_inst.ins.sync_info
        waits = list(si.on_wait) if si is not None and si.on_wait else []
        if len(waits) > 1:
            si.on_wait = waits[:1]
            for i in range(1, len(waits)):
                extra = nc.sync.nop()
                extra.ins.sync_info = _br.SyncInfo(
                    on_wait=waits[i : i + 1], on_update=[]
                )
        nc.all_engine_barrier()
        popped = nc._tile_sem_poison_stack.pop()
        assert popped is self._sem_poison
        assert self.sems is not None
        nc.clear_and_free_semaphores(list(self.sems.allocated().values()))


def _split_multi_waits(nc):
    """Move extra sem waits (>1 per instruction) onto same-engine NOPs
    inserted immediately before the instruction."""
    f = nc.m.functions[0]
    cur_bb = nc.cur_bb
    for block in f.blocks:
        insts = list(block.instructions)
        if not any(
            i.sync_info is not None
            and i.sync_info.on_wait
            and len(i.sync_info.on_wait) > 1
            for i in insts
        ):
            continue
        new_list = []
        for inst in insts:
            si = inst.sync_info
            if si is not None and si.on_wait and len(si.on_wait) > 1:
                waits = list(si.on_wait)
                si.on_wait = waits[:1]
                for w in waits[1:]:
                    eng = nc.engines[inst.engine]
                    nop = eng.nop()
                    tail_bb = cur_bb.bb if hasattr(cur_bb, "bb") else cur_bb
                    tl = list(tail_bb.instructions)
                    assert tl and tl[-1].name == nop.ins.name
                    tail_bb.instructions = tl[:-1]
                    nop.ins.sync_info = _br.SyncInfo(on_wait=[w], on_update=[])
                    new_list.append(nop.ins)
            new_list.append(inst)
        block.instructions = new_list


def _build(SP, trivial_gb):
    """Build the per-core Bass program. SP = padded compacted key count.
    trivial_gb: gamma==1 and beta==0, so the final affine is per-row only."""
    NS = SP // 128  # s-tiles
    NKCH = (SP + 511) // 512  # 512-wide chunks of SP for the K projection

    nc = bass.Bass("TRN2", target_bir_lowering=False, debug=False, num_devices=8)

    p = lambda name, shape, dt: nc.declare_dram_parameter(name, shape, dt, isOutput=False)
    qT = p("qT", [D, T], BF16)
    kT = p("kT", [D, SP], BF16)
    vTs = p("vTs", [NS * D, 128], FP8)  # s-tile-major blocks of v^T
    wqT = p("wqT", [D, DD], BF16)
    wkT = p("wkT", [D, DD], BF16)
    wvT = p("wvT", [D, HPC * (DEPTH + 1)], BF16)
    bq = p("bq", [128, 2], F32)
    bk = p("bk", [128, 2], F32)
    qres = p("qres", [T, DD], BF16)
    gam = p("gamma", [1, DD], BF16)
    bet = p("beta", [1, DD], BF16)
    out = nc.declare_dram_parameter("out", [T, DD], BF16, isOutput=True)

    with _TC(nc) as tc, contextlib.ExitStack() as ctx:
        singles = ctx.enter_context(tc.tile_pool(name="singles", bufs=1))
        persist = ctx.enter_context(tc.tile_pool(name="persist", bufs=1))
        work = ctx.enter_context(tc.tile_pool(name="work", bufs=4))
        dram = ctx.enter_context(tc.tile_pool(name="dram", bufs=1, space="DRAM"))

        # ---- warm the collective path FIRST: the first collective of a NEFF
        # execution pays a ~50-60us setup delay, and Tile serializes every
        # later-emitted DMA transpose behind the preceding collective's
        # completion.  Triggering it at t~0 makes it complete as early as the
        # slowest core's launch allows, so the ctx transposes (needed from
        # ~60us) stall as little as possible.
        warm = singles.tile([128, 1], F32)
        nc.vector.memset(warm[:], 1.0)
        warm_d = dram.tile([128, 1], F32, name="warmd")
        warm_a = dram.tile([128, 1], F32, name="warma")
        nc.sync.dma_start(out=warm_d[:], in_=warm[:])
        nc.gpsimd.collective_compute(
            "AllReduce",
            AluOp.add,
            replica_groups=[[0, 1, 2, 3], [4, 5, 6, 7]],
            ins=[warm_d[:].opt()],
            outs=[warm_a[:].opt()],
        )
        # warm the scalar-engine exp table during the initial DMA wait
        nc.scalar.activation(out=warm[:], in_=warm[:], func=Act.Exp)

        # ---- constants / weights (in consumption order) ----
        wv_sb = singles.tile([128, NKV, HPC * (DEPTH + 1)], BF16)
        nc.sync.dma_start(out=wv_sb[:], in_=wvT[:].rearrange("(kt p) c -> p kt c", p=128))
        bq_sb = singles.tile([128, 2], F32)
        nc.gpsimd.dma_start(out=bq_sb[:], in_=bq[:])
        bk_sb = singles.tile([128, 2], F32)
        nc.gpsimd.dma_start(out=bk_sb[:], in_=bk[:])
        gam_sb = singles.tile([128, DD], BF16)
        g_ap = gam[:]
        nc.gpsimd.dma_start(
            out=gam_sb[:],
            in_=bass.AP(tensor=g_ap.tensor, offset=g_ap.offset, ap=[[0, 128], list(g_ap.ap[-1])]),
        )
        bet_sb = singles.tile([128, DD], BF16)
        b_ap = bet[:]
        nc.gpsimd.dma_start(
            out=bet_sb[:],
            in_=bass.AP(tensor=b_ap.tensor, offset=b_ap.offset, ap=[[0, 128], list(b_ap.ap[-1])]),
        )

        # ---- persistent activations ----
        QT_sb = persist.tile([128, 2, T], BF16)  # [dd-in-tile, ddt, t]
        KT_sb = persist.tile([128, 2, SP], BF16)
        VH_sb = persist.tile([128, NS, HPC * (DEPTH + 1)], BF16)  # [s, st, head*65+c]
        ctxT_sb = persist.tile([128, HPC, T], BF16)  # rows 0..64 valid
        ctxn_sb = persist.tile([128, HPC, TT, 128], BF16)  # transposed ctx
        x_sb = persist.tile([128, TT, DD], BF16)  # residual+ctx, pre-norm
        out_sb = persist.tile([128, TT, DD], BF16)  # normalized output
        mu = singles.tile([128, TT], F32)
        rstd = singles.tile([128, TT], F32)

        # ---- input streaming: all big input DMAs up front on ONE queue in
        # consumption order (vTs, wk, kT, wq, qT, qres).  NB: splitting the
        # inputs across the sync+scalar HWDGE queues was measured WORSE --
        # per-core DMA bandwidth is shared, so a concurrent K/Q stream
        # starves the V operands the first projection is waiting on. ----
        ain = ctx.enter_context(tc.tile_pool(name="ain", bufs=8))
        vin = ctx.enter_context(tc.tile_pool(name="vin", bufs=NS))
        vts = []
        for st in range(NS):
            t_ = vin.tile([128, NKV, 128], FP8, tag="vin", name="vt")
            vts.append(t_)

        def v_dma(st):
            nc.sync.dma_start(
                out=vts[st][:],
                in_=vTs[D * st : D * (st + 1), :].rearrange(
                    "(kt p) s -> p kt s", p=128
                ),
            )

        for st in range(5):
            v_dma(st)
        wk_sb = singles.tile([128, 8, DD], BF16)
        nc.sync.dma_start(out=wk_sb[:], in_=wkT[:].rearrange("(kt p) c -> p kt c", p=128))
        kin = []
        for kt in range(8):
            t_ = ain.tile([128, SP], BF16, tag="kin", name="kin")
            nc.sync.dma_start(out=t_[:], in_=kT[128 * kt : 128 * (kt + 1), :])
            kin.append(t_)
        for st in range(5, NS):
            v_dma(st)
        wq_sb = singles.tile([128, 8, DD], BF16)
        nc.sync.dma_start(out=wq_sb[:], in_=wqT[:].rearrange("(kt p) c -> p kt c", p=128))
        qin = []
        for kt in range(8):
            t_ = ain.tile([128, T], BF16, tag="qin", name="qin")
            nc.sync.dma_start(out=t_[:], in_=qT[128 * kt : 128 * (kt + 1), :])
            qin.append(t_)
        qres_sb = persist.tile([128, TT, DD], BF16)
        nc.sync.dma_start(
            out=qres_sb[:], in_=qres[:].rearrange("(tt p) c -> p tt c", p=128)
        )

        # host-prepared per-key softmax-denominator mask (1.0 for valid keys,
        # 0.0 for compaction padding), DMA'd into VH's den columns after the
        # projection copies (the wv den columns are zero, so PSUM den = 0)
        denm = p("denm", [128, NS * HPC], BF16)
        denm_dst = bass.AP(
            tensor=VH_sb.tensor,
            offset=VH_sb[:, 0, DEPTH].offset,
            ap=[VH_sb.ap[0], [HPC * (DEPTH + 1), NS], [DEPTH + 1, HPC]],
        )

        # ---- V projection (own PSUM scope, closed before attention) ----
        with tc.tile_pool(name="pv", bufs=2, space="PSUM") as pv:
            for st in range(NS):
                ps = pv.tile([128, HPC * (DEPTH + 1)], F32, tag="pv")
                for kt in range(NKV):
                    nc.tensor.matmul(
                        ps[:],
                        vts[st][:, kt, :],
                        wv_sb[:, kt, :],
                        start=(kt == 0),
                        stop=(kt == NKV - 1),
                    )
                if st % 2 == 0:
                    nc.vector.tensor_copy(VH_sb[:, st, :], ps[:])
                else:
                    nc.scalar.copy(VH_sb[:, st, :], ps[:])
        nc.gpsimd.dma_start(out=denm_dst, in_=denm[:])

        # ---- K/Q projections (own PSUM scope) ----
        with tc.tile_pool(name="pqk", bufs=3, space="PSUM") as pqk:

            def k_proj(ddt):
                for sch in range(NKCH):
                    w = min(512, SP - 512 * sch)
                    ps = pqk.tile([128, 512], F32, tag="pqk", name="kps")
                    for kt in range(8):
                        nc.tensor.matmul(
                            ps[:, :w],
                            wk_sb[:, kt, 128 * ddt : 128 * (ddt + 1)],
                            kin[kt][:, 512 * sch : 512 * sch + w],
                            start=(kt == 0),
                            stop=(kt == 7),
                        )
                    dst = KT_sb[:, ddt, 512 * sch : 512 * sch + w]
                    if sch % 2 == 0:
                        nc.vector.tensor_scalar(
                            out=dst, in0=ps[:, :w],
                            scalar1=bk_sb[:, ddt : ddt + 1], scalar2=None,
                            op0=AluOp.add,
                        )
                    else:
                        nc.scalar.activation(
                            out=dst, in_=ps[:, :w], func=Act.Identity,
                            bias=bk_sb[:, ddt : ddt + 1], scale=1.0,
                        )

            def q_proj(ddt):
                for tch in range(4):
                    ps = pqk.tile([128, 512], F32, tag="pqk", name="qps")
                    for kt in range(8):
                        nc.tensor.matmul(
                            ps[:],
                            wq_sb[:, kt, 128 * ddt : 128 * (ddt + 1)],
                            qin[kt][:, 512 * tch : 512 * (tch + 1)],
                            start=(kt == 0),
                            stop=(kt == 7),
                        )
                    dst = QT_sb[:, ddt, 512 * tch : 512 * (tch + 1)]
                    if tch % 2 == 0:
                        nc.scalar.activation(
                            out=dst, in_=ps[:], func=Act.Identity,
                            bias=bq_sb[:, ddt : ddt + 1], scale=1.0,
                        )
                    else:
                        nc.vector.tensor_scalar(
                            out=dst, in0=ps[:],
                            scalar1=bq_sb[:, ddt : ddt + 1], scalar2=None,
                            op0=AluOp.add,
                        )

            k_proj(0)
            q_proj(0)
            k_proj(1)
            q_proj(1)

        # partition-swapped copies of KT/QT: row-group 0-63 holds the odd
        # head's depth rows and 64-127 the even head's, so consecutive
        # s-tiles' K=64 score matmuls can run concurrently in disjoint
        # PE row groups (SBUF->SBUF DMA shifts partitions; engines can't)
        KTd_sb = persist.tile([128, 2, SP], BF16)
        QTd_sb = persist.tile([128, 2, T], BF16)
        for ddt in range(2):
            nc.sync.dma_start(
                out=KTd_sb[64:128, ddt, :], in_=KT_sb[0:64, ddt, :])
            nc.sync.dma_start(
                out=KTd_sb[0:64, ddt, :], in_=KT_sb[64:128, ddt, :])
            nc.sync.dma_start(
                out=QTd_sb[64:128, ddt, :], in_=QT_sb[0:64, ddt, :])
            nc.sync.dma_start(
                out=QTd_sb[0:64, ddt, :], in_=QT_sb[64:128, ddt, :])

        # ---- attention + pipelined LN ----
        stats_dram = dram.tile([128, 32], F32, name="stats")
        ar_dram = dram.tile([128, 32], F32, name="ar")
        epool = ctx.enter_context(tc.tile_pool(name="epool", bufs=6))
        # PSUM: 3 score tiles (2 banks each, depth-3 pipeline) + 1 ctx (2) = 8
        pscore = ctx.enter_context(tc.tile_pool(name="pscore", bufs=3, space="PSUM"))
        pctx = ctx.enter_context(tc.tile_pool(name="pctx", bufs=1, space="PSUM"))

        # exp engine schedule: every act_mod-th tile on the scalar engine
        # (exact), the rest on the vector engine (bit-trick)
        ecnt = [0]

        def emit_exp(e_ap, s_ap, act_mod):
            i = ecnt[0]
            ecnt[0] += 1
            if i % act_mod == act_mod - 1:
                nc.scalar.activation(out=e_ap, in_=s_ap, func=Act.Exp, scale=0.125)
            else:
                nc.vector.tensor_scalar(
                    out=e_ap.bitcast(I16), in0=s_ap,
                    scalar1=ES, scalar2=EB,
                    op0=AluOp.mult, op1=AluOp.add,
                )

        def attn_head(hd, tc_, copy_eng, act_mod=3, fillers=()):
            """Attention for head hd over t-chunk tc_ (1024 queries).

            Software-pipelined: scores(st) and exp(st) are emitted before
            ctx(st-1) so the PE streams scores while ACT/DVE exponentiate
            and the 3-deep score-psum pool absorbs exp latency jitter.
            ``fillers``: closures spread evenly across the head's s-tile
            pair iterations (LN stats work threads through the attention
            instruction stream)."""
            t0 = 1024 * tc_
            ddt, h = hd // 2, hd % 2
            r0 = DEPTH * h
            cps = pctx.tile([DEPTH + 1, 1024], F32, tag="cps", name="cps")
            es = [None] * NS
            fillers = list(fillers)
            npairs = (NS + 1) // 2

            def emit_ctx(st):
                for q2 in range(2):
                    nc.tensor.matmul(
                        cps[:, 512 * q2 : 512 * (q2 + 1)],
                        VH_sb[:, st, (DEPTH + 1) * hd : (DEPTH + 1) * (hd + 1)],
                        es[st][:, 512 * q2 : 512 * (q2 + 1)],
                        start=(st == 0),
                        stop=(st == NS - 1),
                    )

            # odd s-tiles use the partition-swapped K/Q copies so their
            # K=64 score matmuls land in the opposite PE row group and run
            # concurrently with the even s-tile's
            rd0 = DEPTH * (1 - h)
            for it, sp in enumerate(range(0, NS, 2)):
                pair = sp + 1 < NS
                s0 = pscore.tile([128, 1024], F32, tag="sc", name="s")
                s1 = pscore.tile([128, 1024], F32, tag="sc", name="s") if pair else None
                for q2 in range(2):
                    nc.tensor.matmul(
                        s0[:, 512 * q2 : 512 * (q2 + 1)],
                        KT_sb[r0 : r0 + DEPTH, ddt, 128 * sp : 128 * (sp + 1)],
                        QT_sb[r0 : r0 + DEPTH, ddt, t0 + 512 * q2 : t0 + 512 * (q2 + 1)],
                        start=True,
                        stop=True,
                    )
                    if pair:
                        nc.tensor.matmul(
                            s1[:, 512 * q2 : 512 * (q2 + 1)],
                            KTd_sb[rd0 : rd0 + DEPTH, ddt, 128 * (sp + 1) : 128 * (sp + 2)],
                            QTd_sb[rd0 : rd0 + DEPTH, ddt, t0 + 512 * q2 : t0 + 512 * (q2 + 1)],
                            start=True,
                            stop=True,
                        )
                e_ = epool.tile([128, 1024], BF16, tag="e")
                emit_exp(e_[:], s0[:], act_mod)
                es[sp] = e_
                if pair:
                    e2_ = epool.tile([128, 1024], BF16, tag="e")
                    emit_exp(e2_[:], s1[:], act_mod)
                    es[sp + 1] = e2_
                if sp > 0:
                    emit_ctx(sp - 2)
                    if sp - 1 < NS:
                        emit_ctx(sp - 1)
                # spread all fillers evenly over the remaining iterations
                take = -(-len(fillers) // (npairs - it)) if fillers else 0
                for _ in range(take):
                    fillers.pop(0)()
            for st_ in range(2 * ((NS - 1) // 2), NS):
                emit_ctx(st_)
            for f in fillers:
                f()

            dst = ctxT_sb[0 : DEPTH + 1, hd, t0 : t0 + 1024]
            if copy_eng == "v":
                nc.vector.tensor_copy(dst, cps[:])
            else:
                nc.scalar.copy(dst, cps[:])
            nc.sync.dma_start_transpose(
                ctxn_sb[:, hd, 8 * tc_ : 8 * (tc_ + 1), :],
                ctxT_sb[:, hd, t0 : t0 + 1024],
            )

        ln = ctx.enter_context(tc.tile_pool(name="ln", bufs=4))

        hs_t = [
            singles.tile([128, HPC, 16], F32, tag=f"hs{i}", name=f"hs{i}")
            for i in range(2)
        ]
        rinv_t = {}

        def phase5_rinv(tc_, h):
            """Reciprocal of head (tc_, h)'s softmax denominators (vector).
            Must be EMITTED before any phase5_head closures for (tc_, h)."""
            rinv_h = work.tile([128, 8], F32, tag=f"rh{tc_}_{h}", name=f"rh{tc_}_{h}")
            rinv_t[tc_, h] = rinv_h
            sums_ap = bass.AP(
                tensor=ctxn_sb.tensor,
                offset=ctxn_sb[:, h, 8 * tc_, DEPTH].offset,
                ap=[ctxn_sb.ap[0], [128, 8], [1, 1]],
            )
            nc.vector.reciprocal(rinv_h[:], sums_ap)

        def phase5_head(tc_, h, js=range(8), with_rinv=True, sxx_eng="v"):
            """Stats closures for head (tc_, h)'s 64 columns (stats are
            additive over heads).  sxx_eng="s" computes the sum-x^2 half on
            the scalar engine (ACT Square + accum_out)."""
            hs1 = hs_t[tc_]
            cls = []
            if with_rinv:
                cls.append(lambda: phase5_rinv(tc_, h))
            for j in js:
                tt = 8 * tc_ + j
                c0 = DEPTH * h

                def sx(tt=tt, j=j, c0=c0, h=h):
                    nc.vector.scalar_tensor_tensor(
                        out=x_sb[:, tt, c0 : c0 + DEPTH],
                        in0=ctxn_sb[:, h, tt, 0:DEPTH],
                        scalar=rinv_t[tc_, h][:, j : j + 1],
                        in1=qres_sb[:, tt, c0 : c0 + DEPTH],
                        op0=AluOp.mult, op1=AluOp.add,
                        accum_out=hs1[:, h, j : j + 1],
                    )

                def sxx(tt=tt, j=j, c0=c0, h=h):
                    xx = ln.tile([128, DEPTH], BF16, tag="hxx")
                    if sxx_eng == "s":
                        nc.scalar.activation(
                            out=xx[:], in_=x_sb[:, tt, c0 : c0 + DEPTH],
                            func=Act.Square,
                            accum_out=hs1[:, h, 8 + j : 9 + j],
                        )
                    else:
                        nc.vector.scalar_tensor_tensor(
                            out=xx[:], in0=x_sb[:, tt, c0 : c0 + DEPTH], scalar=0.0,
                            in1=x_sb[:, tt, c0 : c0 + DEPTH],
                            op0=AluOp.add, op1=AluOp.mult,
                            accum_out=hs1[:, h, 8 + j : 9 + j],
                        )

                cls += [sx, sxx]
            return cls

        stats32 = singles.tile([128, 32], F32, name="stats32")

        def phase5_merge(tc_):
            """stats32[16*tc_:16*tc_+16] = sum over heads of per-head stats
            (cols 0:8 sum-x, 8:16 sum-x^2)."""
            hs1 = hs_t[tc_]
            merged = bass.AP(
                tensor=hs1.tensor,
                offset=hs1[:, 0, 0].offset,
                ap=[hs1.ap[0], [1, 16], [16, HPC]],
            )
            nc.vector.tensor_reduce(
                out=stats32[:, 16 * tc_ : 16 * tc_ + 16], in_=merged,
                axis=mybir.AxisListType.X, op=AluOp.add,
            )

        def phase5_cc():
            # single 16KB AllReduce for ALL LayerNorm stats; every DMA
            # transpose is emitted before this point so none serializes
            # behind it
            nc.sync.dma_start(out=stats_dram[:, 0:32], in_=stats32[:])
            nc.gpsimd.collective_compute(
                "AllReduce",
                AluOp.add,
                replica_groups=[[0, 1, 2, 3], [4, 5, 6, 7]],
                ins=[stats_dram[:, 0:32].opt()],
                outs=[ar_dram[:, 0:32].opt()],
            )

        def phase7_prep():
            """After the AllReduce: mu and rstd for all 16 t-tiles on the
            vector engine (rsqrt = Quake bit-trick seed + 2 Newton steps)."""
            gst = work.tile([128, 32], F32, tag="gst", name="gst")
            nc.sync.dma_start(out=gst[:], in_=ar_dram[:, 0:32])
            # strided views: sums at cols {0:8, 16:24}, sumsq at {8:16, 24:32}
            sums = bass.AP(tensor=gst.tensor, offset=gst[:, 0].offset,
                           ap=[gst.ap[0], [16, 2], [1, 8]])
            sumsq = bass.AP(tensor=gst.tensor, offset=gst[:, 8].offset,
                            ap=[gst.ap[0], [16, 2], [1, 8]])
            nc.vector.tensor_scalar(
                out=mu[:, 0:16], in0=sums, scalar1=1.0 / D, scalar2=None,
                op0=AluOp.mult,
            )
            var = work.tile([128, 16], F32, tag="var", name="var")
            nc.vector.tensor_tensor(out=var[:], in0=mu[:, 0:16], in1=mu[:, 0:16],
                                    op=AluOp.mult)
            nc.vector.scalar_tensor_tensor(
                out=var[:], in0=sumsq, scalar=1.0 / D, in1=var[:],
                op0=AluOp.mult, op1=AluOp.subtract,
            )
            nc.vector.tensor_scalar(
                out=var[:], in0=var[:], scalar1=LN_EPS, scalar2=None, op0=AluOp.add
            )
            y = work.tile([128, 16], F32, tag="y", name="y")
            nc.vector.tensor_scalar(
                out=y[:].bitcast(mybir.dt.int32),
                in0=var[:].bitcast(mybir.dt.int32),
                scalar1=-0.5, scalar2=float(0x5F3759DF),
                op0=AluOp.mult, op1=AluOp.add,
            )
            tn = work.tile([128, 16], F32, tag="tn", name="tn")
            for it in range(1):
                nc.vector.tensor_tensor(out=tn[:], in0=var[:], in1=y[:], op=AluOp.mult)
                nc.vector.tensor_tensor(out=tn[:], in0=tn[:], in1=y[:], op=AluOp.mult)
                nc.vector.tensor_scalar(
                    out=tn[:], in0=tn[:], scalar1=-0.5, scalar2=1.5,
                    op0=AluOp.mult, op1=AluOp.add,
                )
                nc.vector.tensor_tensor(out=rstd[:, 0:16], in0=y[:], in1=tn[:],
                                        op=AluOp.mult)
            if trivial_gb:
                # out = (x - mu) * rstd needs only a per-row scale+bias:
                # one scalar-engine ACT per tile.  negmurstd = -mu * rstd.
                nc.vector.scalar_tensor_tensor(
                    out=negmurstd[:], in0=mu[:, 0:16], scalar=-1.0,
                    in1=rstd[:, 0:16], op0=AluOp.mult, op1=AluOp.mult,
                )

        negmurstd = work.tile([128, 16], F32, tag="nmr", name="nmr")

        def phase7_g(g):
            """Normalize 4 t-tiles.  With gamma==1/beta==0 the whole affine
            collapses to per-row (x - mu) * rstd: one 2-scalar-op vector
            tensor_scalar OR one scalar-engine ACT per tile, alternated so
            both engines split the tail; otherwise two vector STTs."""
            for k in range(4):
                tt = 4 * g + k
                if trivial_gb:
                    if k % 2 == 0:
                        nc.vector.tensor_scalar(
                            out=out_sb[:, tt, :], in0=x_sb[:, tt, :],
                            scalar1=mu[:, tt : tt + 1],
                            scalar2=rstd[:, tt : tt + 1],
                            op0=AluOp.subtract, op1=AluOp.mult,
                        )
                    else:
                        nc.scalar.activation(
                            out=out_sb[:, tt, :], in_=x_sb[:, tt, :],
                            func=Act.Identity,
                            bias=negmurstd[:, tt : tt + 1],
                            scale=rstd[:, tt : tt + 1],
                        )
                    continue
                s1 = ln.tile([128, DD], BF16, tag="s1")
                nc.vector.scalar_tensor_tensor(
                    out=s1[:], in0=x_sb[:, tt, :], scalar=mu[:, tt : tt + 1],
                    in1=gam_sb[:], op0=AluOp.subtract, op1=AluOp.mult,
                )
                nc.vector.scalar_tensor_tensor(
                    out=out_sb[:, tt, :], in0=s1[:], scalar=rstd[:, tt : tt + 1],
                    in1=bet_sb[:], op0=AluOp.mult, op1=AluOp.add,
                )

        def phase7_out(g):
            nc.sync.dma_start(
                out=out[512 * g : 512 * (g + 1), :].rearrange(
                    "(tt p) c -> p tt c", p=128
                ),
                in_=out_sb[:, 4 * g : 4 * (g + 1), :],
            )

        # Schedule: all 8 heads' LN stats thread through the attention
        # stream on the vector engine.  Chunk-0 stats wait on ctx transposes
        # that are serialized behind the warmup collective (~70-90us local);
        # while those emissions can stall the vector FIFO, every exp routes
        # to the scalar engine (act_mod=1).  A single AllReduce of all stats
        # triggers right after the last head's stats; the tail is prep +
        # scalar-engine normalize + streamed output DMAs.
        attn_head(0, 0, "s", act_mod=2)
        attn_head(1, 0, "v", act_mod=2)
        phase5_rinv(0, 0)
        for cl in phase5_head(0, 0, with_rinv=False):
            cl()
        attn_head(2, 0, "s", act_mod=1)
        phase5_rinv(0, 1)
        for cl in phase5_head(0, 1, with_rinv=False):
            cl()
        attn_head(3, 0, "s", act_mod=1)
        phase5_rinv(0, 2)
        for cl in phase5_head(0, 2, with_rinv=False):
            cl()
        attn_head(0, 1, "s", act_mod=1)
        phase5_rinv(0, 3)
        for cl in phase5_head(0, 3, with_rinv=False):
            cl()
        attn_head(1, 1, "s", act_mod=1)
        phase5_rinv(1, 0)
        for cl in phase5_head(1, 0, with_rinv=False):
            cl()
        attn_head(2, 1, "v", act_mod=2, fillers=[lambda: phase5_merge(0)])
        phase5_rinv(1, 1)
        for cl in phase5_head(1, 1, with_rinv=False):
            cl()
        phase5_rinv(1, 2)
        for cl in phase5_head(1, 2, with_rinv=False):
            cl()
        attn_head(3, 1, "v", act_mod=1)
        phase5_rinv(1, 3)
        for cl in phase5_head(1, 3, with_rinv=False, sxx_eng="s"):
            cl()
        phase5_merge(1)
        phase5_cc()
        phase7_prep()
        for g in range(4):
            phase7_g(g)
            phase7_out(g)

    _split_multi_waits(nc)
    return nc


_CACHE = {}
_LAST_IN_MAPS = None


def kernel(q, k, v, mask, causality, edge_fea, wq, bq, wk, bk, wv, bv, gamma, beta):
    # NB: the reference masks attention row (head eta, batch beta) with
    # mask[eta // 8]; with 4 heads per core this is mask[hg // 2].
    q = np.asarray(q, np.float32)
    k = np.asarray(k, np.float32)
    v = np.asarray(v, np.float32)
    mask = np.asarray(mask)
    wq = np.asarray(wq, np.float32)
    bq = np.asarray(bq, np.float32)
    wk = np.asarray(wk, np.float32)
    bk = np.asarray(bk, np.float32)
    wv = np.asarray(wv, np.float32)
    bv = np.asarray(bv, np.float32)
    gamma = np.asarray(gamma, np.float32)
    beta = np.asarray(beta, np.float32)
    assert int(np.asarray(causality)) == 0

    keep = [np.flatnonzero(mask[g] == 0) for g in range(2)]
    slens = [len(kp) for kp in keep]
    SP = max(128, ((max(slens) + 127) // 128) * 128)
    NS = SP // 128

    qT = [np.ascontiguousarray(q[b].T).astype(BF) for b in range(2)]
    kTc, vTc, denc = {}, {}, {}
    for b in range(2):
        for g in range(2):
            kk = np.zeros((D, SP), BF)
            kk[:, : slens[g]] = k[b][keep[g]].T.astype(BF)
            kTc[b, g] = kk
            vv = np.zeros((D, SP), E4)
            vv[:, : slens[g]] = v[b][keep[g]].T.astype(E4)
            # s-tile-major blocks: [NS, D, 128] -> [NS*D, 128]
            vTc[b, g] = np.ascontiguousarray(
                vv.reshape(D, NS, 128).transpose(1, 0, 2).reshape(NS * D, 128)
            )
    for g in range(2):
        dm = np.zeros((128, NS, HPC), BF)
        valid = (np.arange(SP) < slens[g]).reshape(NS, 128).T  # [128, NS]
        dm[:, :, :] = valid[:, :, None].astype(BF)
        denc[g] = np.ascontiguousarray(dm.reshape(128, NS * HPC))

    in_maps = []
    for c in range(8):
        b, hg = c // 4, c % 4
        g = hg // 2
        c0 = hg * DD
        wvp = np.zeros((D, HPC * (DEPTH + 1)), BF)
        for hh in range(HPC):
            wvp[:, hh * (DEPTH + 1) : hh * (DEPTH + 1) + DEPTH] = (
                wv[c0 + hh * DEPTH : c0 + (hh + 1) * DEPTH].T.astype(BF)
            )
        in_maps.append(
            {
                "qT": qT[b],
                "kT": kTc[b, g],
                "vTs": vTc[b, g],
                "wqT": np.ascontiguousarray(wq[c0 : c0 + DD].T).astype(BF),
                "wkT": np.ascontiguousarray(wk[c0 : c0 + DD].T).astype(BF),
                "wvT": wvp,
                "denm": denc[g],
                "bq": np.ascontiguousarray(bq[c0 : c0 + DD].reshape(2, 128).T),
                "bk": np.ascontiguousarray(bk[c0 : c0 + DD].reshape(2, 128).T),
                "qres": (q[b][:, c0 : c0 + DD] + bv[c0 : c0 + DD]).astype(BF),
                "gamma": gamma[c0 : c0 + DD].reshape(1, DD).astype(BF),
                "beta": beta[c0 : c0 + DD].reshape(1, DD).astype(BF),
            }
        )

    global _LAST_IN_MAPS
    _LAST_IN_MAPS = in_maps
    trivial_gb = bool(np.all(gamma == 1.0) and np.all(beta == 0.0))
    key = (SP, trivial_gb)
    if key not in _CACHE:
        _CACHE[key] = _build(SP, trivial_gb)
    nc = _CACHE[key]

    res = run_bass_kernel_spmd(nc, in_maps, list(range(8))).results

    full = np.empty((B, T, D), np.float32)
    for c in range(8):
        b, hg = c // 4, c % 4
        full[b, :, hg * DD : (hg + 1) * DD] = np.asarray(res[c]["out"], np.float32)
    return full

